# revision 28
# baseline (speedup 1.0000x reference)
"""8-core data-parallel fused attention kernel for TRN2 (Bass/Tile).

Problem: B=8, N=1024 (32x32 grid), DIM=1024, 16 heads x 64, axial RoPE on
first 32 channels of each head, softmax attention, output projection.

Sharding: pure data-parallel -- core b computes batch element b end-to-end.
No collectives.

Design (v8):

- All matmuls bf16 (PSUM f32). QKV computed transposed (features on
  partitions) in per-head-pair blocks: [h_even 64ch; h_odd 64ch] so
  scores are single K=64-contract matmuls at concurrent row bands
  (0,0)/(64,0); attn@V col-packed at (0,0)/(0,64).
- rotate_half = adjacent-partition swap: one DVE stream_shuffle with the
  +-1 signs folded into the host-built sin table (sinF2), final add on
  the otherwise-idle GpSimd. (The earlier pair-swap-matrix PE matmul and
  its PSUM tile are gone.)
- One pipelined attention wave over all 16 (pair, qt) units: round i
  emits unit i scores/exp, then unit i-1 attn@V + den + normalize.
  Measured constraint: scores singles must NOT interleave inside the
  open attn@V accumulation chains -- overlapping 32x32 weight strips
  race the fg/bg weight binding and corrupt results (NaN).
- den = ones-matmul over DVE pairwise pre-sums of the exp tiles. For
  units 8..13 the pre-sums are further tree-summed in-place on GpSimd so
  den is a single matmul-pair; NOT for the last two units -- the gpsimd
  latency lands on the drain path, idles the PE >3.4us, and HAM
  re-throttles to 1.2 GHz for the proj tail (measured, ~8us cold).
- PSUM: scores 2x[128,1024] double-buffer (4 banks) + attn@V out (1) +
  shared ps_mm ring (3) holding QKV/V/proj chains AND den tiles; the
  third buffer decouples consecutive QKV chains from the DVE psA
  consumption (chain-start WAR stalls, measured 375-460ns deltas).
- Input DMAs: one coarse 3D dispatch per tensor region, sync queue ONLY
  (dispatches carry DGE credit waits; on the scalar queue they block the
  exp stream -- measured 7.7us PE stall).
- proj(qt0) emitted inside the wave from round 8 (when every qt0 norm is
  already emitted -- earlier emission deadlocks the in-order PE queue;
  later emission starves rounds 8-9, measured). proj(qt1) trails the
  wave. Output bf16, 4-way DMA split per tile so the last drain doesn't
  gate the fixed epilogue.

Measured on trn2 (8 cores, axon): HW exec 258.7us, rel err 5.0e-3.
Prior: v2 baseline 271.5us, v1 ~347us. Fixed overheads inside the
measured window: ~8us runtime lead-in before the first instruction,
~7.6us semaphore-sweep epilogue after the last output DMA.
"""

import os
import sys

for _p in ("/opt/trn_rl_repo",):
    if os.path.isdir(_p) and _p not in sys.path:
        sys.path.insert(0, _p)

import numpy as np
import ml_dtypes

import concourse.bass as bass
import concourse.bacc as bacc
import concourse.mybir as mybir
import concourse.tile as tile
from concourse.bass_utils import run_bass_kernel_spmd

P = 128
NTOK = 1024
DIM = 1024
HEADS = 16
HD = 64
ROT = 32
QT = 512          # free-dim tile for matmuls (one PSUM bank of f32)
NQ = NTOK // QT   # 2
NPAIR = 8
BF = mybir.dt.bfloat16
F32 = mybir.dt.float32
AL = mybir.AluOpType
AF = mybir.ActivationFunctionType

LAST_RESULT = None
_BUILT = None


# ---------------------------------------------------------------- host prep

def _axial_tables():
    """cos/sin[t, d] for t=0..1023 (t=h*32+w), d=0..31, exactly as reference."""
    rot_half = 8
    base = np.linspace(1.0, 512.0, rot_half) * np.pi          # (8,)
    th = np.linspace(-1.0, 1.0, 32)[:, None] * base[None, :]  # (32, 8)
    fh = np.repeat(th, 2, axis=-1)                            # (32, 16)
    freqs = np.zeros((32, 32, ROT))
    freqs[:, :, :16] = fh[:, None, :]                         # H-axis channels
    freqs[:, :, 16:] = fh[None, :, :]                         # W-axis channels
    f = freqs.reshape(NTOK, ROT)
    return np.cos(f).astype(np.float32), np.sin(f).astype(np.float32)


def _prep_weights(Wqkv, Wproj, bproj):
    Wq, Wk, Wv = Wqkv[0:DIM], Wqkv[DIM:2 * DIM], Wqkv[2 * DIM:3 * DIM]
    # per-pair feature blocks: [h_even 64ch; h_odd 64ch] for Q then K.
    blocks = []
    for pr in range(NPAIR):
        for W in (Wq, Wk):
            blocks.append(W[2 * pr * HD:(2 * pr + 2) * HD])   # (128, 1024)
    wqk = np.concatenate(blocks, axis=0)                      # (2048, 1024)

    cos_td, sin_td = _axial_tables()                          # (1024, 32)
    cosF = np.ones((P, NTOK), np.float32)
    sinF = np.zeros((P, NTOK), np.float32)
    cosF[0:32] = cos_td.T
    cosF[64:96] = cos_td.T
    sinF[0:32] = sin_td.T
    sinF[64:96] = sin_td.T
    # rotate_half via DVE stream_shuffle (adjacent-partition swap): the
    # destination sign (-1 on even rows) is folded into the source table,
    # sinF2[j] = sinF[j] * (+1 if j even else -1), so after the swap
    # us[i] = u2[i^1] = sign_i * u[i^1]. Pass rows stay 0.
    sinF2 = sinF.copy()
    sinF2[1::2] *= -1.0

    biasT = bproj.reshape(8, P).T.copy()                      # (128, 8)
    bf = ml_dtypes.bfloat16
    return {
        "wqk": np.ascontiguousarray(wqk.T).astype(bf),        # (1024, 2048)
        "wv": np.ascontiguousarray(Wv.T).astype(bf),          # (1024, 1024)
        "wp": np.ascontiguousarray(Wproj.T).astype(bf),       # (1024, 1024)
        "cosf": np.ascontiguousarray(cosF).astype(bf),
        "sinf": np.ascontiguousarray(sinF2).astype(bf),
        "biasT": np.ascontiguousarray(biasT.astype(np.float32)),
    }


# ------------------------------------------------------------- bass builder

def _build():
    nc = bacc.Bacc()
    xT_e = nc.declare_dram_parameter("xT", [DIM, NTOK], BF, isOutput=False)
    wqk_e = nc.declare_dram_parameter("wqk", [DIM, 2 * DIM], BF, isOutput=False)
    wv_e = nc.declare_dram_parameter("wv", [DIM, DIM], BF, isOutput=False)
    wp_e = nc.declare_dram_parameter("wp", [DIM, DIM], BF, isOutput=False)
    cos_e = nc.declare_dram_parameter("cosf", [P, NTOK], BF, isOutput=False)
    sin_e = nc.declare_dram_parameter("sinf", [P, NTOK], BF, isOutput=False)
    b_e = nc.declare_dram_parameter("biasT", [P, 8], F32, isOutput=False)
    out_e = nc.declare_dram_parameter("out", [DIM, NTOK], BF, isOutput=True)

    with tile.TileContext(nc) as tc:
        with (
            tc.tile_pool(name="persist", bufs=1) as persist,
            tc.tile_pool(name="work", bufs=3) as work,
            tc.tile_pool(name="work3", bufs=12) as work3,
            tc.tile_pool(name="ps_sc", bufs=2, space="PSUM") as ps_sc_pool,
            tc.tile_pool(name="ps_av", bufs=1, space="PSUM") as ps_av_pool,
            tc.tile_pool(name="ps_mm", bufs=3, space="PSUM") as ps_mm_pool,
        ):
            xT = persist.tile([P, 8, NTOK], BF)
            wqk = persist.tile([P, 8, 2 * DIM], BF)
            wv = persist.tile([P, 8, DIM], BF)
            wp = persist.tile([P, 8, DIM], BF)
            cosF = persist.tile([P, NTOK], BF)
            sinF = persist.tile([P, NTOK], BF)
            ones64 = persist.tile([P, HD], BF)
            biasT = persist.tile([P, 8], F32)
            # roped QK, pair-stacked: partitions = [rot_e, pass_e, rot_o,
            # pass_o], chunk = pair index
            q2 = persist.tile([P, NPAIR, NTOK], BF)
            k2 = persist.tile([P, NPAIR, NTOK], BF)
            # V natural: [k-token partitions, kc, head*64+d]
            v = persist.tile([P, 8, DIM], BF)
            # attention out, transposed: partition 64*(h%2)+d, chunk h//2
            outT = persist.tile([P, 8, NTOK], BF)

            # ---------------- PE warmup: the clock ramps 0.65->2.4 GHz only
            # after ~3us of continuous busy. Spin dependency-free matmuls on
            # memset scratch during the DMA lead-in so real work starts at
            # full clock. Result is never read.
            wup = persist.tile([P, QT], BF)
            nc.vector.memset(wup[:], 0.5)
            pw = ps_mm_pool.tile([P, QT], F32, tag="ps_mm")
            for wi in range(20):
                nc.tensor.matmul(pw[:], wup[:, 0:P], wup[:],
                                 start=(wi == 0), stop=(wi == 19))

            # ---------------- input DMAs, ordered by first use. One coarse
            # 3D dispatch per region (all 8 cc chunks at once): per-chunk
            # dispatch (~600ns each on the in-order sync queue, with DGE
            # credit waits) serialized the input load; and DMAs must stay
            # off the scalar queue, where they block the exp stream.
            def load3d(dst, src_e, c0, c1):
                # dst[:, cc, c0:c1] <- src_e[cc*P:(cc+1)*P, c0:c1] for all cc
                nc.sync.dma_start(
                    out=dst[:, :, c0:c1],
                    in_=src_e.rearrange("(c p) n -> p c n", p=P)[:, :, c0:c1])

            load3d(wqk, wqk_e, 0, 2 * P)
            load3d(xT, xT_e, 0, QT)
            nc.sync.dma_start(out=cosF[:], in_=cos_e[:, :])
            nc.sync.dma_start(out=sinF[:], in_=sin_e[:, :])
            load3d(xT, xT_e, QT, NTOK)
            load3d(wqk, wqk_e, 2 * P, 4 * P)
            load3d(wv, wv_e, 0, QT)
            load3d(wqk, wqk_e, 4 * P, 8 * P)
            load3d(wv, wv_e, QT, DIM)
            load3d(wqk, wqk_e, 8 * P, 16 * P)
            nc.sync.dma_start(out=biasT[:], in_=b_e[:, :])
            load3d(wp, wp_e, 0, DIM)
            nc.vector.memset(ones64[:], 1.0)

            # ---------------- QKV^T + RoPE epilogue. rotate_half is a pure
            # adjacent-partition swap (signs pre-folded into sinF): one DVE
            # stream_shuffle instead of a PE matmul through PSUM. The final
            # add runs on the idle GpSimd engine except for pairs 0-1, whose
            # outputs gate the start of the attention wave.
            SWAP_MASK = [i ^ 1 for i in range(32)]

            def qkv_stream(pairs, t2_outer=False):
                if t2_outer:
                    order = [(pr, t2) for t2 in range(NQ) for pr in pairs]
                else:
                    order = [(pr, t2) for pr in pairs for t2 in range(NQ)]
                for pr, t2 in order:
                    for which in range(2):
                        blk = 2 * pr + which
                        dst = q2 if which == 0 else k2
                        ts_ = slice(t2 * QT, (t2 + 1) * QT)
                        psA = ps_mm_pool.tile([P, QT], F32, tag="ps_mm")
                        for cc in range(8):
                            nc.tensor.matmul(
                                psA[:],
                                wqk[:, cc, blk * P:(blk + 1) * P],
                                xT[:, cc, ts_],
                                start=(cc == 0), stop=(cc == 7))
                        yield
                        t1 = work.tile([P, QT], BF, tag="t1")
                        u2 = work.tile([P, QT], BF, tag="u")
                        us = work.tile([P, QT], BF, tag="us")
                        nc.vector.tensor_tensor(
                            t1[:], psA[:], cosF[:, ts_], op=AL.mult)
                        nc.vector.tensor_tensor(
                            u2[:], psA[:], sinF[:, ts_], op=AL.mult)
                        nc.vector.stream_shuffle(us[:], u2[:], SWAP_MASK)
                        nc.gpsimd.tensor_add(dst[:, pr, ts_], t1[:], us[:])
                        yield

            # ---------------- V = x @ Wv^T, natural orientation
            def v_units(g):
                for tt in range(8):
                    pt = ps_mm_pool.tile([P, QT], F32, tag="ps_mm")
                    for cc in range(8):
                        nc.tensor.matmul(
                            pt[:],
                            xT[:, cc, tt * P:(tt + 1) * P],
                            wv[:, cc, g * QT:(g + 1) * QT],
                            start=(cc == 0), stop=(cc == 7))
                    nc.vector.tensor_copy(v[:, tt, g * QT:(g + 1) * QT], pt[:])
                    yield

            # ---------------- pipelined attention wave. Round i runs unit
            # i's scores/exp interleaved per-kc with unit i-1's attn@V
            # chains: every scores LDW gets the previous MM's drain window
            # (fixing the measured 312/224ns LDW-serialization alternation)
            # and attnV needs no separate filler. den (ones-matmul over the
            # DVE pairwise pre-sums) uses the same col-band positions as
            # attn@V, so it is emitted only after those chains close --
            # interleaving two open accumulation chains at one tile
            # position corrupts PSUM (measured). post_round emits deferred
            # work (proj qt0 units) after a given round's den/norm, which
            # is the earliest emission point that cannot deadlock the
            # in-order PE queue on a later norm.
            def attn_wave(units, post_round=None):
                prev = None
                for rnd, item in enumerate(list(units) + [None]):
                    cur = None
                    if item is not None:
                        pr, qt = item
                        # late units (qt=1 rounds): gpsimd is idle once the
                        # qkv filler is exhausted, so sum the 4 DVE pre-sums
                        # down to 1 tile there (in-place adds) and emit a
                        # single den matmul-pair instead of four.
                        cur = {"pr": pr,
                               "qs": slice(qt * QT, (qt + 1) * QT),
                               "aTs": [], "sums": None,
                               # not for the last two units: their den falls
                               # in the wave's drain, where the serial tree
                               # adds would sit on the proj(1) critical path.
                               "tree": rnd < 2 * NPAIR - 2}
                    if prev is not None:
                        po = ps_av_pool.tile([P, QT], F32, tag="ps_po")
                        # den shares the ps_mm ring (3 bufs): the extra
                        # buffer also decouples consecutive QKV chains from
                        # the DVE's psA consumption (chain-start WAR stalls).
                        den = ps_mm_pool.tile([P, QT], F32, tag="ps_mm")
                        ph0 = 2 * prev["pr"]
                        ph1 = ph0 + 1
                    for kc in range(8):
                        if cur is not None:
                            ks = slice(kc * P, (kc + 1) * P)
                            psS = ps_sc_pool.tile([P, 2 * QT], F32, tag="ps_s")
                            nc.tensor.matmul(
                                psS[:, 0:QT], k2[0:HD, cur["pr"], ks],
                                q2[0:HD, cur["pr"], cur["qs"]],
                                start=True, stop=True, tile_position=(0, 0))
                            nc.tensor.matmul(
                                psS[:, QT:2 * QT], k2[HD:P, cur["pr"], ks],
                                q2[HD:P, cur["pr"], cur["qs"]],
                                start=True, stop=True, tile_position=(64, 0))
                            aT = work3.tile([P, 2 * QT], BF, tag="aT", bufs=12)
                            nc.scalar.activation(aT[:], psS[:], AF.Exp,
                                                 scale=0.125)
                            cur["aTs"].append(aT)
                        if kc % 2 == 1:
                            yield
                    if prev is not None:
                        for kc in range(8):
                            st = (kc == 0)
                            sp = (kc == 7)
                            paT = prev["aTs"][kc]
                            nc.tensor.matmul(
                                po[0:HD, :], v[:, kc, ph0 * HD:(ph0 + 1) * HD],
                                paT[:, 0:QT], start=st, stop=sp,
                                tile_position=(0, 0))
                            nc.tensor.matmul(
                                po[HD:P, :], v[:, kc, ph1 * HD:(ph1 + 1) * HD],
                                paT[:, QT:2 * QT], start=st, stop=sp,
                                tile_position=(0, 64))
                            if kc % 2 == 1:
                                yield
                    if cur is not None:
                        # mid-qt0 rounds are filler-heavy: the QKV epilogue
                        # mults queue behind these pre-sums on the DVE and
                        # stall the next QKV chain's PSUM reuse. GpSimd has
                        # slack there; units 0-1 (startup) and 6-7 (pair-7
                        # RoPE-add latency shares the gpsimd queue) stay on
                        # the DVE.
                        sum_eng = nc.gpsimd if 2 <= rnd <= 5 else nc.vector
                        sums = []
                        for j in range(4):
                            sm = work3.tile([P, 2 * QT], BF, tag="aTs", bufs=8)
                            sum_eng.tensor_add(
                                sm[:], cur["aTs"][2 * j][:],
                                cur["aTs"][2 * j + 1][:])
                            sums.append(sm)
                        cur["sums"] = sums
                        if cur["tree"]:
                            # qt0 rounds are PE-bound with DVE slack (the
                            # pre-sums moved to gpsimd there); qt1 rounds
                            # are the reverse.
                            te = nc.vector if rnd < NPAIR else nc.gpsimd
                            te.tensor_add(
                                sums[0][:], sums[0][:], sums[1][:])
                            te.tensor_add(
                                sums[2][:], sums[2][:], sums[3][:])
                            te.tensor_add(
                                sums[0][:], sums[0][:], sums[2][:])
                    if prev is not None:
                        if prev["tree"]:
                            sm = prev["sums"][0]
                            nc.tensor.matmul(
                                den[0:HD, :], ones64[:], sm[:, 0:QT],
                                start=True, stop=True, tile_position=(0, 0))
                            nc.tensor.matmul(
                                den[HD:P, :], ones64[:], sm[:, QT:2 * QT],
                                start=True, stop=True, tile_position=(0, 64))
                        else:
                            for j, sm in enumerate(prev["sums"]):
                                st = (j == 0)
                                sp = (j == 3)
                                nc.tensor.matmul(
                                    den[0:HD, :], ones64[:], sm[:, 0:QT],
                                    start=st, stop=sp, tile_position=(0, 0))
                                nc.tensor.matmul(
                                    den[HD:P, :], ones64[:], sm[:, QT:2 * QT],
                                    start=st, stop=sp, tile_position=(0, 64))
                        yield
                        rd = work.tile([P, QT], F32, tag="rd")
                        nc.vector.reciprocal_approx_fast(rd[:], den[:])
                        nc.vector.tensor_tensor(
                            outT[:, prev["pr"], prev["qs"]], po[:], rd[:],
                            op=AL.mult)
                        yield
                    if post_round is not None:
                        for _ in post_round(rnd):
                            yield
                    prev = cur

            # ---------------- output projection + bias
            def proj_units(qt):
                qs = slice(qt * QT, (qt + 1) * QT)
                for ot in range(8):
                    os_ = slice(ot * P, (ot + 1) * P)
                    pt = ps_mm_pool.tile([P, QT], F32, tag="ps_mm")
                    for cc in range(8):
                        nc.tensor.matmul(
                            pt[:], wp[:, cc, os_], outT[:, cc, qs],
                            start=(cc == 0), stop=(cc == 7))
                    # bf16 output (harness casts back; error budget 2e-2
                    # dwarfs the 0.4% quantization) halves the output-DMA
                    # bytes; 4-way split shortens the final drain that gates
                    # the fixed sem-cleanup epilogue.
                    ys = work.tile([P, QT], BF, tag="ys")
                    nc.vector.tensor_scalar_add(ys[:], pt[:], biasT[:, ot:ot + 1])
                    for sp in range(4):
                        rows = slice(sp * 32, (sp + 1) * 32)
                        nc.sync.dma_start(
                            out=out_e[ot * P + sp * 32:ot * P + (sp + 1) * 32, qs],
                            in_=ys[rows])
                    yield

            def run(gen):
                for _ in gen:
                    pass

            def weave(a, b, ra=2, rb=1):
                """Generator: alternate ra units from a with rb units from b."""
                a, b = iter(a), iter(b)
                alive_a = alive_b = True
                while alive_a or alive_b:
                    for _ in range(ra):
                        if alive_a:
                            try:
                                next(a)
                            except StopIteration:
                                alive_a = False
                            else:
                                yield
                    for _ in range(rb):
                        if alive_b:
                            try:
                                next(b)
                            except StopIteration:
                                alive_b = False
                            else:
                                yield

            def chain(*gens):
                for g in gens:
                    for _ in g:
                        yield

            # schedule: qkv pairs 0-1 woven with V(g0) up front; one
            # pipelined attention wave over all 16 (pair, qt) units, woven
            # with the remaining qkv + V(g1) as PE filler. proj(0) units
            # are emitted inside the wave via post_round once every qt=0
            # norm has been emitted (rounds 10-17); proj(1) trails.
            run(weave(qkv_stream([0, 1]), v_units(0), 8, 4))
            filler = chain(qkv_stream([2, 3, 4]), v_units(1),
                           qkv_stream([5, 6, 7]))
            proj0 = [proj_units(0)]

            def post_round(rnd):
                # norm(pair 7, qt0) is emitted in round 8's den/norm
                # section, before this hook runs -- so proj(0) chains are
                # emission-safe from round 8 on.
                if rnd >= 8:
                    try:
                        next(proj0[0])
                    except StopIteration:
                        return
                    yield

            units = ([(pr, 0) for pr in range(NPAIR)]
                     + [(pr, 1) for pr in range(NPAIR)])
            run(weave(attn_wave(units, post_round), filler, 6, 7))
            run(proj0[0])
            run(proj_units(1))

    nc.compile()
    return nc


def _get_nc():
    global _BUILT
    if _BUILT is None:
        _BUILT = _build()
    return _BUILT


# ------------------------------------------------- tracing support (axon)

def _ensure_trace_hooks():
    """Register the NTFF profile hook that the bare agent image's antenv
    stub lacks, and neuter the artifact upload (no bucket in-container)."""
    import types
    import concourse.bass_utils as bu

    bu.upload_artifacts = lambda tmpdir: f"local:{tmpdir}"
    try:
        from antenv.axon_hooks import get_axon_ntff_profile_hook  # noqa: F401
        return
    except ImportError:
        pass
    mod = types.ModuleType("antenv.axon_hooks")
    _state = {"hook": None}
    mod.set_axon_ntff_profile_hook = lambda h: _state.__setitem__("hook", h)
    mod.get_axon_ntff_profile_hook = lambda: _state["hook"]
    import antenv
    sys.modules["antenv.axon_hooks"] = mod
    antenv.axon_hooks = mod
    try:
        from trn_agent_boot.trn_boot import _ntff_profile_via_ctypes
        hook = _ntff_profile_via_ctypes("/opt/axon/libaxon_pjrt.so")
        if hook is not None:
            mod.set_axon_ntff_profile_hook(hook)
    except Exception as e:  # pragma: no cover
        print(f"NTFF hook install failed: {e!r}")


# ----------------------------------------------------------------- kernel()

def kernel(x, Wqkv, Wproj, bproj):
    global LAST_RESULT
    x = np.asarray(x, np.float32)
    Wqkv = np.asarray(Wqkv, np.float32)
    Wproj = np.asarray(Wproj, np.float32)
    bproj = np.asarray(bproj, np.float32)
    B = x.shape[0]

    base = _prep_weights(Wqkv, Wproj, bproj)
    bf = ml_dtypes.bfloat16
    in_maps = [
        dict(base, xT=np.ascontiguousarray(x[b].T).astype(bf)) for b in range(B)
    ]
    nc = _get_nc()
    trace = bool(os.environ.get("KBENCH_TRACE"))
    if trace:
        _ensure_trace_hooks()
    res = run_bass_kernel_spmd(
        nc, in_maps, core_ids=list(range(B)), trace=trace)
    LAST_RESULT = res
    out = np.stack([np.asarray(res.results[b]["out"], np.float32).T
                    for b in range(B)])
    return np.ascontiguousarray(out.astype(np.float32))



# revision 29
# speedup vs baseline: 1.0614x; 1.0614x over previous
"""8-core data-parallel fused attention kernel for TRN2 (Bass/Tile).

Problem: B=8, N=1024 (32x32 grid), DIM=1024, 16 heads x 64, axial RoPE on
first 32 channels of each head, softmax attention, output projection.

Sharding: pure data-parallel -- core b computes batch element b end-to-end.
No collectives.

Design (v8):

- All matmuls bf16 (PSUM f32). QKV computed transposed (features on
  partitions) in per-head-pair blocks: [h_even 64ch; h_odd 64ch] so
  scores are single K=64-contract matmuls at concurrent row bands
  (0,0)/(64,0); attn@V col-packed at (0,0)/(0,64).
- rotate_half = adjacent-partition swap: one DVE stream_shuffle with the
  +-1 signs folded into the host-built sin table (sinF2), final add on
  the otherwise-idle GpSimd. (The earlier pair-swap-matrix PE matmul and
  its PSUM tile are gone.)
- One pipelined attention wave over all 16 (pair, qt) units: round i
  emits unit i scores/exp, then unit i-1 attn@V + den + normalize.
  Measured constraint: scores singles must NOT interleave inside the
  open attn@V accumulation chains -- overlapping 32x32 weight strips
  race the fg/bg weight binding and corrupt results (NaN).
- den = ones-matmul over DVE pairwise pre-sums of the exp tiles. For
  units 8..13 the pre-sums are further tree-summed in-place on GpSimd so
  den is a single matmul-pair; NOT for the last two units -- the gpsimd
  latency lands on the drain path, idles the PE >3.4us, and HAM
  re-throttles to 1.2 GHz for the proj tail (measured, ~8us cold).
- PSUM: scores 2x[128,1024] double-buffer (4 banks) + attn@V out (1) +
  shared ps_mm ring (3) holding QKV/V/proj chains AND den tiles; the
  third buffer decouples consecutive QKV chains from the DVE psA
  consumption (chain-start WAR stalls, measured 375-460ns deltas).
- Input DMAs: one coarse 3D dispatch per tensor region, sync queue ONLY
  (dispatches carry DGE credit waits; on the scalar queue they block the
  exp stream -- measured 7.7us PE stall).
- proj(qt0) emitted inside the wave from round 8 (when every qt0 norm is
  already emitted -- earlier emission deadlocks the in-order PE queue;
  later emission starves rounds 8-9, measured). proj(qt1) trails the
  wave. Output bf16, 4-way DMA split per tile so the last drain doesn't
  gate the fixed epilogue.

Measured on trn2 (8 cores, axon): HW exec 258.7us, rel err 5.0e-3.
Prior: v2 baseline 271.5us, v1 ~347us. Fixed overheads inside the
measured window: ~8us runtime lead-in before the first instruction,
~7.6us semaphore-sweep epilogue after the last output DMA.
"""

import os
import sys

for _p in ("/opt/trn_rl_repo",):
    if os.path.isdir(_p) and _p not in sys.path:
        sys.path.insert(0, _p)

import numpy as np
import ml_dtypes

import concourse.bass as bass
import concourse.bacc as bacc
import concourse.mybir as mybir
import concourse.tile as tile
from concourse.bass_utils import run_bass_kernel_spmd

P = 128
NTOK = 1024
DIM = 1024
HEADS = 16
HD = 64
ROT = 32
QT = 512          # free-dim tile for matmuls (one PSUM bank of f32)
NQ = NTOK // QT   # 2
NPAIR = 8
BF = mybir.dt.bfloat16
F32 = mybir.dt.float32
AL = mybir.AluOpType
AF = mybir.ActivationFunctionType

LAST_RESULT = None
_BUILT = None


# ---------------------------------------------------------------- host prep

def _axial_tables():
    """cos/sin[t, d] for t=0..1023 (t=h*32+w), d=0..31, exactly as reference."""
    rot_half = 8
    base = np.linspace(1.0, 512.0, rot_half) * np.pi          # (8,)
    th = np.linspace(-1.0, 1.0, 32)[:, None] * base[None, :]  # (32, 8)
    fh = np.repeat(th, 2, axis=-1)                            # (32, 16)
    freqs = np.zeros((32, 32, ROT))
    freqs[:, :, :16] = fh[:, None, :]                         # H-axis channels
    freqs[:, :, 16:] = fh[None, :, :]                         # W-axis channels
    f = freqs.reshape(NTOK, ROT)
    return np.cos(f).astype(np.float32), np.sin(f).astype(np.float32)


def _prep_weights(Wqkv, Wproj, bproj):
    Wq, Wk, Wv = Wqkv[0:DIM], Wqkv[DIM:2 * DIM], Wqkv[2 * DIM:3 * DIM]
    # per-pair feature blocks: [h_even 64ch; h_odd 64ch] for Q then K.
    blocks = []
    for pr in range(NPAIR):
        for W in (Wq, Wk):
            blocks.append(W[2 * pr * HD:(2 * pr + 2) * HD])   # (128, 1024)
    wqk = np.concatenate(blocks, axis=0)                      # (2048, 1024)

    cos_td, sin_td = _axial_tables()                          # (1024, 32)
    cosF = np.ones((P, NTOK), np.float32)
    sinF = np.zeros((P, NTOK), np.float32)
    cosF[0:32] = cos_td.T
    cosF[64:96] = cos_td.T
    sinF[0:32] = sin_td.T
    sinF[64:96] = sin_td.T
    # rotate_half via DVE stream_shuffle (adjacent-partition swap): the
    # destination sign (-1 on even rows) is folded into the source table,
    # sinF2[j] = sinF[j] * (+1 if j even else -1), so after the swap
    # us[i] = u2[i^1] = sign_i * u[i^1]. Pass rows stay 0.
    sinF2 = sinF.copy()
    sinF2[1::2] *= -1.0

    biasT = bproj.reshape(8, P).T.copy()                      # (128, 8)
    bf = ml_dtypes.bfloat16
    return {
        "wqk": np.ascontiguousarray(wqk.T).astype(bf),        # (1024, 2048)
        "wv": np.ascontiguousarray(Wv.T).astype(bf),          # (1024, 1024)
        "wp": np.ascontiguousarray(Wproj.T).astype(bf),       # (1024, 1024)
        "cosf": np.ascontiguousarray(cosF).astype(bf),
        "sinf": np.ascontiguousarray(sinF2).astype(bf),
        "biasT": np.ascontiguousarray(biasT.astype(np.float32)),
    }


# ------------------------------------------------------------- bass builder

def _build():
    nc = bacc.Bacc()
    xT_e = nc.declare_dram_parameter("xT", [DIM, NTOK], BF, isOutput=False)
    wqk_e = nc.declare_dram_parameter("wqk", [DIM, 2 * DIM], BF, isOutput=False)
    wv_e = nc.declare_dram_parameter("wv", [DIM, DIM], BF, isOutput=False)
    wp_e = nc.declare_dram_parameter("wp", [DIM, DIM], BF, isOutput=False)
    cos_e = nc.declare_dram_parameter("cosf", [P, NTOK], BF, isOutput=False)
    sin_e = nc.declare_dram_parameter("sinf", [P, NTOK], BF, isOutput=False)
    b_e = nc.declare_dram_parameter("biasT", [P, 8], F32, isOutput=False)
    out_e = nc.declare_dram_parameter("out", [DIM, NTOK], BF, isOutput=True)

    with tile.TileContext(nc) as tc:
        with (
            tc.tile_pool(name="persist", bufs=1) as persist,
            tc.tile_pool(name="work", bufs=3) as work,
            tc.tile_pool(name="work3", bufs=12) as work3,
            tc.tile_pool(name="ps_sc", bufs=2, space="PSUM") as ps_sc_pool,
            tc.tile_pool(name="ps_av", bufs=1, space="PSUM") as ps_av_pool,
            tc.tile_pool(name="ps_mm", bufs=3, space="PSUM") as ps_mm_pool,
        ):
            xT = persist.tile([P, 8, NTOK], BF)
            wqk = persist.tile([P, 8, 2 * DIM], BF)
            wv = persist.tile([P, 8, DIM], BF)
            wp = persist.tile([P, 8, DIM], BF)
            cosF = persist.tile([P, NTOK], BF)
            sinF = persist.tile([P, NTOK], BF)
            ones64 = persist.tile([P, HD], BF)
            biasT = persist.tile([P, 8], F32)
            # roped QK, pair-stacked: partitions = [rot_e, pass_e, rot_o,
            # pass_o], chunk = pair index
            q2 = persist.tile([P, NPAIR, NTOK], BF)
            k2 = persist.tile([P, NPAIR, NTOK], BF)
            # V natural: [k-token partitions, kc, head*64+d]
            v = persist.tile([P, 8, DIM], BF)
            # attention out, transposed: partition 64*(h%2)+d, chunk h//2
            outT = persist.tile([P, 8, NTOK], BF)

            # ---------------- PE warmup: the clock ramps 0.65->2.4 GHz only
            # after ~3us of continuous busy. Spin dependency-free matmuls on
            # memset scratch during the DMA lead-in so real work starts at
            # full clock. Result is never read.
            wup = persist.tile([P, QT], BF)
            nc.vector.memset(wup[:], 0.5)
            pw = ps_mm_pool.tile([P, QT], F32, tag="ps_mm")
            for wi in range(20):
                nc.tensor.matmul(pw[:], wup[:, 0:P], wup[:],
                                 start=(wi == 0), stop=(wi == 19))

            # ---------------- input DMAs, ordered by first use. One coarse
            # 3D dispatch per region (all 8 cc chunks at once): per-chunk
            # dispatch (~600ns each on the in-order sync queue, with DGE
            # credit waits) serialized the input load; and DMAs must stay
            # off the scalar queue, where they block the exp stream.
            def load3d(dst, src_e, c0, c1):
                # dst[:, cc, c0:c1] <- src_e[cc*P:(cc+1)*P, c0:c1] for all cc
                nc.sync.dma_start(
                    out=dst[:, :, c0:c1],
                    in_=src_e.rearrange("(c p) n -> p c n", p=P)[:, :, c0:c1])

            load3d(wqk, wqk_e, 0, 2 * P)
            load3d(xT, xT_e, 0, QT)
            nc.sync.dma_start(out=cosF[:], in_=cos_e[:, :])
            nc.sync.dma_start(out=sinF[:], in_=sin_e[:, :])
            load3d(xT, xT_e, QT, NTOK)
            load3d(wqk, wqk_e, 2 * P, 4 * P)
            load3d(wv, wv_e, 0, QT)
            load3d(wqk, wqk_e, 4 * P, 8 * P)
            load3d(wv, wv_e, QT, DIM)
            load3d(wqk, wqk_e, 8 * P, 16 * P)
            nc.sync.dma_start(out=biasT[:], in_=b_e[:, :])
            load3d(wp, wp_e, 0, DIM)
            nc.vector.memset(ones64[:], 1.0)

            # ---------------- QKV^T + RoPE epilogue. rotate_half is a pure
            # adjacent-partition swap (signs pre-folded into sinF): one DVE
            # stream_shuffle instead of a PE matmul through PSUM. The final
            # add runs on the idle GpSimd engine except for pairs 0-1, whose
            # outputs gate the start of the attention wave.
            SWAP_MASK = [i ^ 1 for i in range(32)]

            def qkv_stream(pairs, t2_outer=False):
                if t2_outer:
                    order = [(pr, t2) for t2 in range(NQ) for pr in pairs]
                else:
                    order = [(pr, t2) for pr in pairs for t2 in range(NQ)]
                for pr, t2 in order:
                    for which in range(2):
                        blk = 2 * pr + which
                        dst = q2 if which == 0 else k2
                        ts_ = slice(t2 * QT, (t2 + 1) * QT)
                        psA = ps_mm_pool.tile([P, QT], F32, tag="ps_mm")
                        for cc in range(8):
                            nc.tensor.matmul(
                                psA[:],
                                wqk[:, cc, blk * P:(blk + 1) * P],
                                xT[:, cc, ts_],
                                start=(cc == 0), stop=(cc == 7))
                        yield
                        t1 = work.tile([P, QT], BF, tag="t1")
                        u2 = work.tile([P, QT], BF, tag="u")
                        us = work.tile([P, QT], BF, tag="us")
                        nc.vector.tensor_tensor(
                            t1[:], psA[:], cosF[:, ts_], op=AL.mult)
                        nc.vector.tensor_tensor(
                            u2[:], psA[:], sinF[:, ts_], op=AL.mult)
                        nc.vector.stream_shuffle(us[:], u2[:], SWAP_MASK)
                        nc.gpsimd.tensor_add(dst[:, pr, ts_], t1[:], us[:])
                        yield

            # ---------------- V = x @ Wv^T, natural orientation
            def v_units(g):
                for tt in range(8):
                    pt = ps_mm_pool.tile([P, QT], F32, tag="ps_mm")
                    for cc in range(8):
                        nc.tensor.matmul(
                            pt[:],
                            xT[:, cc, tt * P:(tt + 1) * P],
                            wv[:, cc, g * QT:(g + 1) * QT],
                            start=(cc == 0), stop=(cc == 7))
                    nc.vector.tensor_copy(v[:, tt, g * QT:(g + 1) * QT], pt[:])
                    yield

            # ---------------- pipelined attention wave. Round i runs unit
            # i's scores/exp interleaved per-kc with unit i-1's attn@V
            # chains: every scores LDW gets the previous MM's drain window
            # (fixing the measured 312/224ns LDW-serialization alternation)
            # and attnV needs no separate filler. den (ones-matmul over the
            # DVE pairwise pre-sums) uses the same col-band positions as
            # attn@V, so it is emitted only after those chains close --
            # interleaving two open accumulation chains at one tile
            # position corrupts PSUM (measured). post_round emits deferred
            # work (proj qt0 units) after a given round's den/norm, which
            # is the earliest emission point that cannot deadlock the
            # in-order PE queue on a later norm.
            def attn_wave(units, post_round=None):
                prev = None
                for rnd, item in enumerate(list(units) + [None]):
                    cur = None
                    if item is not None:
                        pr, qt = item
                        # late units (qt=1 rounds): gpsimd is idle once the
                        # qkv filler is exhausted, so sum the 4 DVE pre-sums
                        # down to 1 tile there (in-place adds) and emit a
                        # single den matmul-pair instead of four.
                        cur = {"pr": pr,
                               "qs": slice(qt * QT, (qt + 1) * QT),
                               "aTs": [], "sums": None,
                               # not for the last two units: their den falls
                               # in the wave's drain, where the serial gpsimd
                               # adds would sit on the proj(1) critical path.
                               "tree": NPAIR <= rnd < 2 * NPAIR - 2}
                    if prev is not None:
                        po = ps_av_pool.tile([P, QT], F32, tag="ps_po")
                        # den shares the ps_mm ring (3 bufs): the extra
                        # buffer also decouples consecutive QKV chains from
                        # the DVE's psA consumption (chain-start WAR stalls).
                        den = ps_mm_pool.tile([P, QT], F32, tag="ps_mm")
                        ph0 = 2 * prev["pr"]
                        ph1 = ph0 + 1
                    for kc in range(8):
                        if cur is not None:
                            ks = slice(kc * P, (kc + 1) * P)
                            psS = ps_sc_pool.tile([P, 2 * QT], F32, tag="ps_s")
                            nc.tensor.matmul(
                                psS[:, 0:QT], k2[0:HD, cur["pr"], ks],
                                q2[0:HD, cur["pr"], cur["qs"]],
                                start=True, stop=True, tile_position=(0, 0))
                            nc.tensor.matmul(
                                psS[:, QT:2 * QT], k2[HD:P, cur["pr"], ks],
                                q2[HD:P, cur["pr"], cur["qs"]],
                                start=True, stop=True, tile_position=(64, 0))
                            aT = work3.tile([P, 2 * QT], BF, tag="aT", bufs=12)
                            nc.scalar.activation(aT[:], psS[:], AF.Exp,
                                                 scale=0.125)
                            cur["aTs"].append(aT)
                        if kc % 2 == 1:
                            yield
                    if prev is not None:
                        for kc in range(8):
                            st = (kc == 0)
                            sp = (kc == 7)
                            paT = prev["aTs"][kc]
                            nc.tensor.matmul(
                                po[0:HD, :], v[:, kc, ph0 * HD:(ph0 + 1) * HD],
                                paT[:, 0:QT], start=st, stop=sp,
                                tile_position=(0, 0))
                            nc.tensor.matmul(
                                po[HD:P, :], v[:, kc, ph1 * HD:(ph1 + 1) * HD],
                                paT[:, QT:2 * QT], start=st, stop=sp,
                                tile_position=(0, 64))
                            if kc % 2 == 1:
                                yield
                    if cur is not None:
                        sums = []
                        for j in range(4):
                            sm = work3.tile([P, 2 * QT], BF, tag="aTs", bufs=8)
                            nc.vector.tensor_add(
                                sm[:], cur["aTs"][2 * j][:],
                                cur["aTs"][2 * j + 1][:])
                            sums.append(sm)
                        cur["sums"] = sums
                        if cur["tree"]:
                            nc.gpsimd.tensor_add(
                                sums[0][:], sums[0][:], sums[1][:])
                            nc.gpsimd.tensor_add(
                                sums[2][:], sums[2][:], sums[3][:])
                            nc.gpsimd.tensor_add(
                                sums[0][:], sums[0][:], sums[2][:])
                    if prev is not None:
                        if prev["tree"]:
                            sm = prev["sums"][0]
                            nc.tensor.matmul(
                                den[0:HD, :], ones64[:], sm[:, 0:QT],
                                start=True, stop=True, tile_position=(0, 0))
                            nc.tensor.matmul(
                                den[HD:P, :], ones64[:], sm[:, QT:2 * QT],
                                start=True, stop=True, tile_position=(0, 64))
                        else:
                            for j, sm in enumerate(prev["sums"]):
                                st = (j == 0)
                                sp = (j == 3)
                                nc.tensor.matmul(
                                    den[0:HD, :], ones64[:], sm[:, 0:QT],
                                    start=st, stop=sp, tile_position=(0, 0))
                                nc.tensor.matmul(
                                    den[HD:P, :], ones64[:], sm[:, QT:2 * QT],
                                    start=st, stop=sp, tile_position=(0, 64))
                        yield
                        rd = work.tile([P, QT], F32, tag="rd")
                        nc.vector.reciprocal_approx_fast(rd[:], den[:])
                        nc.vector.tensor_tensor(
                            outT[:, prev["pr"], prev["qs"]], po[:], rd[:],
                            op=AL.mult)
                        yield
                    if post_round is not None:
                        for _ in post_round(rnd):
                            yield
                    prev = cur

            # ---------------- output projection + bias
            def proj_units(qt):
                qs = slice(qt * QT, (qt + 1) * QT)
                for ot in range(8):
                    os_ = slice(ot * P, (ot + 1) * P)
                    pt = ps_mm_pool.tile([P, QT], F32, tag="ps_mm")
                    for cc in range(8):
                        nc.tensor.matmul(
                            pt[:], wp[:, cc, os_], outT[:, cc, qs],
                            start=(cc == 0), stop=(cc == 7))
                    # bf16 output (harness casts back; error budget 2e-2
                    # dwarfs the 0.4% quantization) halves the output-DMA
                    # bytes; 4-way split shortens the final drain that gates
                    # the fixed sem-cleanup epilogue.
                    ys = work.tile([P, QT], BF, tag="ys")
                    nc.vector.tensor_scalar_add(ys[:], pt[:], biasT[:, ot:ot + 1])
                    for sp in range(4):
                        rows = slice(sp * 32, (sp + 1) * 32)
                        nc.sync.dma_start(
                            out=out_e[ot * P + sp * 32:ot * P + (sp + 1) * 32, qs],
                            in_=ys[rows])
                    yield

            def run(gen):
                for _ in gen:
                    pass

            def weave(a, b, ra=2, rb=1):
                """Generator: alternate ra units from a with rb units from b."""
                a, b = iter(a), iter(b)
                alive_a = alive_b = True
                while alive_a or alive_b:
                    for _ in range(ra):
                        if alive_a:
                            try:
                                next(a)
                            except StopIteration:
                                alive_a = False
                            else:
                                yield
                    for _ in range(rb):
                        if alive_b:
                            try:
                                next(b)
                            except StopIteration:
                                alive_b = False
                            else:
                                yield

            def chain(*gens):
                for g in gens:
                    for _ in g:
                        yield

            # schedule: qkv pairs 0-1 woven with V(g0) up front; one
            # pipelined attention wave over all 16 (pair, qt) units, woven
            # with the remaining qkv + V(g1) as PE filler. proj(0) units
            # are emitted inside the wave via post_round once every qt=0
            # norm has been emitted (rounds 10-17); proj(1) trails.
            run(weave(qkv_stream([0, 1]), v_units(0), 8, 4))
            filler = chain(qkv_stream([2, 3, 4]), v_units(1),
                           qkv_stream([5, 6, 7]))
            proj0 = [proj_units(0)]

            def post_round(rnd):
                # norm(pair 7, qt0) is emitted in round 8's den/norm
                # section, before this hook runs -- so proj(0) chains are
                # emission-safe from round 8 on.
                if rnd >= 8:
                    try:
                        next(proj0[0])
                    except StopIteration:
                        return
                    yield

            units = ([(pr, 0) for pr in range(NPAIR)]
                     + [(pr, 1) for pr in range(NPAIR)])
            run(weave(attn_wave(units, post_round), filler, 6, 7))
            run(proj0[0])
            run(proj_units(1))

    nc.compile()
    return nc


def _get_nc():
    global _BUILT
    if _BUILT is None:
        _BUILT = _build()
    return _BUILT


# ------------------------------------------------- tracing support (axon)

def _ensure_trace_hooks():
    """Register the NTFF profile hook that the bare agent image's antenv
    stub lacks, and neuter the artifact upload (no bucket in-container)."""
    import types
    import concourse.bass_utils as bu

    bu.upload_artifacts = lambda tmpdir: f"local:{tmpdir}"
    try:
        from antenv.axon_hooks import get_axon_ntff_profile_hook  # noqa: F401
        return
    except ImportError:
        pass
    mod = types.ModuleType("antenv.axon_hooks")
    _state = {"hook": None}
    mod.set_axon_ntff_profile_hook = lambda h: _state.__setitem__("hook", h)
    mod.get_axon_ntff_profile_hook = lambda: _state["hook"]
    import antenv
    sys.modules["antenv.axon_hooks"] = mod
    antenv.axon_hooks = mod
    try:
        from trn_agent_boot.trn_boot import _ntff_profile_via_ctypes
        hook = _ntff_profile_via_ctypes("/opt/axon/libaxon_pjrt.so")
        if hook is not None:
            mod.set_axon_ntff_profile_hook(hook)
    except Exception as e:  # pragma: no cover
        print(f"NTFF hook install failed: {e!r}")


# ----------------------------------------------------------------- kernel()

def kernel(x, Wqkv, Wproj, bproj):
    global LAST_RESULT
    x = np.asarray(x, np.float32)
    Wqkv = np.asarray(Wqkv, np.float32)
    Wproj = np.asarray(Wproj, np.float32)
    bproj = np.asarray(bproj, np.float32)
    B = x.shape[0]

    base = _prep_weights(Wqkv, Wproj, bproj)
    bf = ml_dtypes.bfloat16
    in_maps = [
        dict(base, xT=np.ascontiguousarray(x[b].T).astype(bf)) for b in range(B)
    ]
    nc = _get_nc()
    trace = bool(os.environ.get("KBENCH_TRACE"))
    if trace:
        _ensure_trace_hooks()
    res = run_bass_kernel_spmd(
        nc, in_maps, core_ids=list(range(B)), trace=trace)
    LAST_RESULT = res
    out = np.stack([np.asarray(res.results[b]["out"], np.float32).T
                    for b in range(B)])
    return np.ascontiguousarray(out.astype(np.float32))



# revision 30
# speedup vs baseline: 1.0629x; 1.0013x over previous
"""8-core data-parallel fused attention kernel for TRN2 (Bass/Tile).

Problem: B=8, N=1024 (32x32 grid), DIM=1024, 16 heads x 64, axial RoPE on
first 32 channels of each head, softmax attention, output projection.

Sharding: pure data-parallel -- core b computes batch element b end-to-end.
No collectives.

Design (v8):

- All matmuls bf16 (PSUM f32). QKV computed transposed (features on
  partitions) in per-head-pair blocks: [h_even 64ch; h_odd 64ch] so
  scores are single K=64-contract matmuls at concurrent row bands
  (0,0)/(64,0); attn@V col-packed at (0,0)/(0,64).
- rotate_half = adjacent-partition swap: one DVE stream_shuffle with the
  +-1 signs folded into the host-built sin table (sinF2), final add on
  the otherwise-idle GpSimd. (The earlier pair-swap-matrix PE matmul and
  its PSUM tile are gone.)
- One pipelined attention wave over all 16 (pair, qt) units: round i
  emits unit i scores/exp, then unit i-1 attn@V + den + normalize.
  Measured constraint: scores singles must NOT interleave inside the
  open attn@V accumulation chains -- overlapping 32x32 weight strips
  race the fg/bg weight binding and corrupt results (NaN).
- den = ones-matmul over DVE pairwise pre-sums of the exp tiles. For
  units 8..13 the pre-sums are further tree-summed in-place on GpSimd so
  den is a single matmul-pair; NOT for the last two units -- the gpsimd
  latency lands on the drain path, idles the PE >3.4us, and HAM
  re-throttles to 1.2 GHz for the proj tail (measured, ~8us cold).
- PSUM: scores 2x[128,1024] double-buffer (4 banks) + attn@V out (1) +
  shared ps_mm ring (3) holding QKV/V/proj chains AND den tiles; the
  third buffer decouples consecutive QKV chains from the DVE psA
  consumption (chain-start WAR stalls, measured 375-460ns deltas).
- Input DMAs: one coarse 3D dispatch per tensor region, sync queue ONLY
  (dispatches carry DGE credit waits; on the scalar queue they block the
  exp stream -- measured 7.7us PE stall).
- proj(qt0) emitted inside the wave from round 8 (when every qt0 norm is
  already emitted -- earlier emission deadlocks the in-order PE queue;
  later emission starves rounds 8-9, measured). proj(qt1) trails the
  wave. Output bf16, 4-way DMA split per tile so the last drain doesn't
  gate the fixed epilogue.

Measured on trn2 (8 cores, axon): HW exec 258.7us, rel err 5.0e-3.
Prior: v2 baseline 271.5us, v1 ~347us. Fixed overheads inside the
measured window: ~8us runtime lead-in before the first instruction,
~7.6us semaphore-sweep epilogue after the last output DMA.
"""

import os
import sys

for _p in ("/opt/trn_rl_repo",):
    if os.path.isdir(_p) and _p not in sys.path:
        sys.path.insert(0, _p)

import numpy as np
import ml_dtypes

import concourse.bass as bass
import concourse.bacc as bacc
import concourse.mybir as mybir
import concourse.tile as tile
from concourse.bass_utils import run_bass_kernel_spmd

P = 128
NTOK = 1024
DIM = 1024
HEADS = 16
HD = 64
ROT = 32
QT = 512          # free-dim tile for matmuls (one PSUM bank of f32)
NQ = NTOK // QT   # 2
NPAIR = 8
BF = mybir.dt.bfloat16
F32 = mybir.dt.float32
AL = mybir.AluOpType
AF = mybir.ActivationFunctionType

LAST_RESULT = None
_BUILT = None


# ---------------------------------------------------------------- host prep

def _axial_tables():
    """cos/sin[t, d] for t=0..1023 (t=h*32+w), d=0..31, exactly as reference."""
    rot_half = 8
    base = np.linspace(1.0, 512.0, rot_half) * np.pi          # (8,)
    th = np.linspace(-1.0, 1.0, 32)[:, None] * base[None, :]  # (32, 8)
    fh = np.repeat(th, 2, axis=-1)                            # (32, 16)
    freqs = np.zeros((32, 32, ROT))
    freqs[:, :, :16] = fh[:, None, :]                         # H-axis channels
    freqs[:, :, 16:] = fh[None, :, :]                         # W-axis channels
    f = freqs.reshape(NTOK, ROT)
    return np.cos(f).astype(np.float32), np.sin(f).astype(np.float32)


def _prep_weights(Wqkv, Wproj, bproj):
    Wq, Wk, Wv = Wqkv[0:DIM], Wqkv[DIM:2 * DIM], Wqkv[2 * DIM:3 * DIM]
    # per-pair feature blocks: [h_even 64ch; h_odd 64ch] for Q then K.
    blocks = []
    for pr in range(NPAIR):
        for W in (Wq, Wk):
            blocks.append(W[2 * pr * HD:(2 * pr + 2) * HD])   # (128, 1024)
    wqk = np.concatenate(blocks, axis=0)                      # (2048, 1024)

    cos_td, sin_td = _axial_tables()                          # (1024, 32)
    cosF = np.ones((P, NTOK), np.float32)
    sinF = np.zeros((P, NTOK), np.float32)
    cosF[0:32] = cos_td.T
    cosF[64:96] = cos_td.T
    sinF[0:32] = sin_td.T
    sinF[64:96] = sin_td.T
    # rotate_half via DVE stream_shuffle (adjacent-partition swap): the
    # destination sign (-1 on even rows) is folded into the source table,
    # sinF2[j] = sinF[j] * (+1 if j even else -1), so after the swap
    # us[i] = u2[i^1] = sign_i * u[i^1]. Pass rows stay 0.
    sinF2 = sinF.copy()
    sinF2[1::2] *= -1.0

    biasT = bproj.reshape(8, P).T.copy()                      # (128, 8)
    bf = ml_dtypes.bfloat16
    return {
        "wqk": np.ascontiguousarray(wqk.T).astype(bf),        # (1024, 2048)
        "wv": np.ascontiguousarray(Wv.T).astype(bf),          # (1024, 1024)
        "wp": np.ascontiguousarray(Wproj.T).astype(bf),       # (1024, 1024)
        "cosf": np.ascontiguousarray(cosF).astype(bf),
        "sinf": np.ascontiguousarray(sinF2).astype(bf),
        "biasT": np.ascontiguousarray(biasT.astype(np.float32)),
    }


# ------------------------------------------------------------- bass builder

def _build():
    nc = bacc.Bacc()
    xT_e = nc.declare_dram_parameter("xT", [DIM, NTOK], BF, isOutput=False)
    wqk_e = nc.declare_dram_parameter("wqk", [DIM, 2 * DIM], BF, isOutput=False)
    wv_e = nc.declare_dram_parameter("wv", [DIM, DIM], BF, isOutput=False)
    wp_e = nc.declare_dram_parameter("wp", [DIM, DIM], BF, isOutput=False)
    cos_e = nc.declare_dram_parameter("cosf", [P, NTOK], BF, isOutput=False)
    sin_e = nc.declare_dram_parameter("sinf", [P, NTOK], BF, isOutput=False)
    b_e = nc.declare_dram_parameter("biasT", [P, 8], F32, isOutput=False)
    out_e = nc.declare_dram_parameter("out", [DIM, NTOK], BF, isOutput=True)

    with tile.TileContext(nc) as tc:
        with (
            tc.tile_pool(name="persist", bufs=1) as persist,
            tc.tile_pool(name="work", bufs=3) as work,
            tc.tile_pool(name="work3", bufs=12) as work3,
            tc.tile_pool(name="ps_sc", bufs=2, space="PSUM") as ps_sc_pool,
            tc.tile_pool(name="ps_av", bufs=1, space="PSUM") as ps_av_pool,
            tc.tile_pool(name="ps_mm", bufs=3, space="PSUM") as ps_mm_pool,
        ):
            xT = persist.tile([P, 8, NTOK], BF)
            wqk = persist.tile([P, 8, 2 * DIM], BF)
            wv = persist.tile([P, 8, DIM], BF)
            wp = persist.tile([P, 8, DIM], BF)
            cosF = persist.tile([P, NTOK], BF)
            sinF = persist.tile([P, NTOK], BF)
            ones64 = persist.tile([P, HD], BF)
            biasT = persist.tile([P, 8], F32)
            # roped QK, pair-stacked: partitions = [rot_e, pass_e, rot_o,
            # pass_o], chunk = pair index
            q2 = persist.tile([P, NPAIR, NTOK], BF)
            k2 = persist.tile([P, NPAIR, NTOK], BF)
            # V natural: [k-token partitions, kc, head*64+d]
            v = persist.tile([P, 8, DIM], BF)
            # attention out, transposed: partition 64*(h%2)+d, chunk h//2
            outT = persist.tile([P, 8, NTOK], BF)

            # ---------------- PE warmup: the clock ramps 0.65->2.4 GHz only
            # after ~3us of continuous busy. Spin dependency-free matmuls on
            # memset scratch during the DMA lead-in so real work starts at
            # full clock. Result is never read.
            wup = persist.tile([P, QT], BF)
            nc.vector.memset(wup[:], 0.5)
            pw = ps_mm_pool.tile([P, QT], F32, tag="ps_mm")
            for wi in range(20):
                nc.tensor.matmul(pw[:], wup[:, 0:P], wup[:],
                                 start=(wi == 0), stop=(wi == 19))

            # ---------------- input DMAs, ordered by first use. One coarse
            # 3D dispatch per region (all 8 cc chunks at once): per-chunk
            # dispatch (~600ns each on the in-order sync queue, with DGE
            # credit waits) serialized the input load; and DMAs must stay
            # off the scalar queue, where they block the exp stream.
            def load3d(dst, src_e, c0, c1):
                # dst[:, cc, c0:c1] <- src_e[cc*P:(cc+1)*P, c0:c1] for all cc
                nc.sync.dma_start(
                    out=dst[:, :, c0:c1],
                    in_=src_e.rearrange("(c p) n -> p c n", p=P)[:, :, c0:c1])

            load3d(wqk, wqk_e, 0, 2 * P)
            load3d(xT, xT_e, 0, QT)
            nc.sync.dma_start(out=cosF[:], in_=cos_e[:, :])
            nc.sync.dma_start(out=sinF[:], in_=sin_e[:, :])
            load3d(xT, xT_e, QT, NTOK)
            load3d(wqk, wqk_e, 2 * P, 4 * P)
            load3d(wv, wv_e, 0, QT)
            load3d(wqk, wqk_e, 4 * P, 8 * P)
            load3d(wv, wv_e, QT, DIM)
            load3d(wqk, wqk_e, 8 * P, 16 * P)
            nc.sync.dma_start(out=biasT[:], in_=b_e[:, :])
            load3d(wp, wp_e, 0, DIM)
            nc.vector.memset(ones64[:], 1.0)

            # ---------------- QKV^T + RoPE epilogue. rotate_half is a pure
            # adjacent-partition swap (signs pre-folded into sinF): one DVE
            # stream_shuffle instead of a PE matmul through PSUM. The final
            # add runs on the idle GpSimd engine except for pairs 0-1, whose
            # outputs gate the start of the attention wave.
            SWAP_MASK = [i ^ 1 for i in range(32)]

            def qkv_stream(pairs, t2_outer=False):
                if t2_outer:
                    order = [(pr, t2) for t2 in range(NQ) for pr in pairs]
                else:
                    order = [(pr, t2) for pr in pairs for t2 in range(NQ)]
                for pr, t2 in order:
                    for which in range(2):
                        blk = 2 * pr + which
                        dst = q2 if which == 0 else k2
                        ts_ = slice(t2 * QT, (t2 + 1) * QT)
                        psA = ps_mm_pool.tile([P, QT], F32, tag="ps_mm")
                        for cc in range(8):
                            nc.tensor.matmul(
                                psA[:],
                                wqk[:, cc, blk * P:(blk + 1) * P],
                                xT[:, cc, ts_],
                                start=(cc == 0), stop=(cc == 7))
                        yield
                        t1 = work.tile([P, QT], BF, tag="t1")
                        u2 = work.tile([P, QT], BF, tag="u")
                        us = work.tile([P, QT], BF, tag="us")
                        nc.vector.tensor_tensor(
                            t1[:], psA[:], cosF[:, ts_], op=AL.mult)
                        nc.vector.tensor_tensor(
                            u2[:], psA[:], sinF[:, ts_], op=AL.mult)
                        nc.vector.stream_shuffle(us[:], u2[:], SWAP_MASK)
                        nc.gpsimd.tensor_add(dst[:, pr, ts_], t1[:], us[:])
                        yield

            # ---------------- V = x @ Wv^T, natural orientation
            def v_units(g):
                for tt in range(8):
                    pt = ps_mm_pool.tile([P, QT], F32, tag="ps_mm")
                    for cc in range(8):
                        nc.tensor.matmul(
                            pt[:],
                            xT[:, cc, tt * P:(tt + 1) * P],
                            wv[:, cc, g * QT:(g + 1) * QT],
                            start=(cc == 0), stop=(cc == 7))
                    nc.vector.tensor_copy(v[:, tt, g * QT:(g + 1) * QT], pt[:])
                    yield

            # ---------------- pipelined attention wave. Round i runs unit
            # i's scores/exp interleaved per-kc with unit i-1's attn@V
            # chains: every scores LDW gets the previous MM's drain window
            # (fixing the measured 312/224ns LDW-serialization alternation)
            # and attnV needs no separate filler. den (ones-matmul over the
            # DVE pairwise pre-sums) uses the same col-band positions as
            # attn@V, so it is emitted only after those chains close --
            # interleaving two open accumulation chains at one tile
            # position corrupts PSUM (measured). post_round emits deferred
            # work (proj qt0 units) after a given round's den/norm, which
            # is the earliest emission point that cannot deadlock the
            # in-order PE queue on a later norm.
            def attn_wave(units, post_round=None):
                prev = None
                for rnd, item in enumerate(list(units) + [None]):
                    cur = None
                    if item is not None:
                        pr, qt = item
                        # late units (qt=1 rounds): gpsimd is idle once the
                        # qkv filler is exhausted, so sum the 4 DVE pre-sums
                        # down to 1 tile there (in-place adds) and emit a
                        # single den matmul-pair instead of four.
                        cur = {"pr": pr,
                               "qs": slice(qt * QT, (qt + 1) * QT),
                               "aTs": [], "sums": None,
                               # not for the last two units: their den falls
                               # in the wave's drain, where the serial gpsimd
                               # adds would sit on the proj(1) critical path.
                               "tree": NPAIR <= rnd < 2 * NPAIR - 2}
                    if prev is not None:
                        po = ps_av_pool.tile([P, QT], F32, tag="ps_po")
                        # den shares the ps_mm ring (3 bufs): the extra
                        # buffer also decouples consecutive QKV chains from
                        # the DVE's psA consumption (chain-start WAR stalls).
                        den = ps_mm_pool.tile([P, QT], F32, tag="ps_mm")
                        ph0 = 2 * prev["pr"]
                        ph1 = ph0 + 1
                    for kc in range(8):
                        if cur is not None:
                            ks = slice(kc * P, (kc + 1) * P)
                            psS = ps_sc_pool.tile([P, 2 * QT], F32, tag="ps_s")
                            nc.tensor.matmul(
                                psS[:, 0:QT], k2[0:HD, cur["pr"], ks],
                                q2[0:HD, cur["pr"], cur["qs"]],
                                start=True, stop=True, tile_position=(0, 0))
                            nc.tensor.matmul(
                                psS[:, QT:2 * QT], k2[HD:P, cur["pr"], ks],
                                q2[HD:P, cur["pr"], cur["qs"]],
                                start=True, stop=True, tile_position=(64, 0))
                            aT = work3.tile([P, 2 * QT], BF, tag="aT", bufs=12)
                            nc.scalar.activation(aT[:], psS[:], AF.Exp,
                                                 scale=0.125)
                            cur["aTs"].append(aT)
                        if kc % 2 == 1:
                            yield
                    if prev is not None:
                        for kc in range(8):
                            st = (kc == 0)
                            sp = (kc == 7)
                            paT = prev["aTs"][kc]
                            nc.tensor.matmul(
                                po[0:HD, :], v[:, kc, ph0 * HD:(ph0 + 1) * HD],
                                paT[:, 0:QT], start=st, stop=sp,
                                tile_position=(0, 0))
                            nc.tensor.matmul(
                                po[HD:P, :], v[:, kc, ph1 * HD:(ph1 + 1) * HD],
                                paT[:, QT:2 * QT], start=st, stop=sp,
                                tile_position=(0, 64))
                            if kc % 2 == 1:
                                yield
                    if cur is not None:
                        sums = []
                        for j in range(4):
                            sm = work3.tile([P, 2 * QT], BF, tag="aTs", bufs=8)
                            nc.vector.tensor_add(
                                sm[:], cur["aTs"][2 * j][:],
                                cur["aTs"][2 * j + 1][:])
                            sums.append(sm)
                        cur["sums"] = sums
                        if cur["tree"]:
                            nc.gpsimd.tensor_add(
                                sums[0][:], sums[0][:], sums[1][:])
                            nc.gpsimd.tensor_add(
                                sums[2][:], sums[2][:], sums[3][:])
                            # final level on the DVE: qt1 rounds carry no
                            # filler DVE work, and the den matmul was
                            # measured waiting ~1us on the serial gpsimd
                            # chain when all three adds ran there.
                            nc.vector.tensor_add(
                                sums[0][:], sums[0][:], sums[2][:])
                    if prev is not None:
                        if prev["tree"]:
                            sm = prev["sums"][0]
                            nc.tensor.matmul(
                                den[0:HD, :], ones64[:], sm[:, 0:QT],
                                start=True, stop=True, tile_position=(0, 0))
                            nc.tensor.matmul(
                                den[HD:P, :], ones64[:], sm[:, QT:2 * QT],
                                start=True, stop=True, tile_position=(0, 64))
                        else:
                            for j, sm in enumerate(prev["sums"]):
                                st = (j == 0)
                                sp = (j == 3)
                                nc.tensor.matmul(
                                    den[0:HD, :], ones64[:], sm[:, 0:QT],
                                    start=st, stop=sp, tile_position=(0, 0))
                                nc.tensor.matmul(
                                    den[HD:P, :], ones64[:], sm[:, QT:2 * QT],
                                    start=st, stop=sp, tile_position=(0, 64))
                        yield
                        rd = work.tile([P, QT], F32, tag="rd")
                        nc.vector.reciprocal_approx_fast(rd[:], den[:])
                        nc.vector.tensor_tensor(
                            outT[:, prev["pr"], prev["qs"]], po[:], rd[:],
                            op=AL.mult)
                        yield
                    if post_round is not None:
                        for _ in post_round(rnd):
                            yield
                    prev = cur

            # ---------------- output projection + bias
            def proj_units(qt):
                qs = slice(qt * QT, (qt + 1) * QT)
                for ot in range(8):
                    os_ = slice(ot * P, (ot + 1) * P)
                    pt = ps_mm_pool.tile([P, QT], F32, tag="ps_mm")
                    for cc in range(8):
                        nc.tensor.matmul(
                            pt[:], wp[:, cc, os_], outT[:, cc, qs],
                            start=(cc == 0), stop=(cc == 7))
                    # bf16 output (harness casts back; error budget 2e-2
                    # dwarfs the 0.4% quantization) halves the output-DMA
                    # bytes; 4-way split shortens the final drain that gates
                    # the fixed sem-cleanup epilogue.
                    ys = work.tile([P, QT], BF, tag="ys")
                    nc.vector.tensor_scalar_add(ys[:], pt[:], biasT[:, ot:ot + 1])
                    for sp in range(4):
                        rows = slice(sp * 32, (sp + 1) * 32)
                        nc.sync.dma_start(
                            out=out_e[ot * P + sp * 32:ot * P + (sp + 1) * 32, qs],
                            in_=ys[rows])
                    yield

            def run(gen):
                for _ in gen:
                    pass

            def weave(a, b, ra=2, rb=1):
                """Generator: alternate ra units from a with rb units from b."""
                a, b = iter(a), iter(b)
                alive_a = alive_b = True
                while alive_a or alive_b:
                    for _ in range(ra):
                        if alive_a:
                            try:
                                next(a)
                            except StopIteration:
                                alive_a = False
                            else:
                                yield
                    for _ in range(rb):
                        if alive_b:
                            try:
                                next(b)
                            except StopIteration:
                                alive_b = False
                            else:
                                yield

            def chain(*gens):
                for g in gens:
                    for _ in g:
                        yield

            # schedule: qkv pairs 0-1 woven with V(g0) up front; one
            # pipelined attention wave over all 16 (pair, qt) units, woven
            # with the remaining qkv + V(g1) as PE filler. proj(0) units
            # are emitted inside the wave via post_round once every qt=0
            # norm has been emitted (rounds 10-17); proj(1) trails.
            run(weave(qkv_stream([0, 1]), v_units(0), 8, 4))
            filler = chain(qkv_stream([2, 3, 4]), v_units(1),
                           qkv_stream([5, 6, 7]))
            proj0 = [proj_units(0)]

            def post_round(rnd):
                # norm(pair 7, qt0) is emitted in round 8's den/norm
                # section, before this hook runs -- so proj(0) chains are
                # emission-safe from round 8 on.
                if rnd >= 8:
                    try:
                        next(proj0[0])
                    except StopIteration:
                        return
                    yield

            units = ([(pr, 0) for pr in range(NPAIR)]
                     + [(pr, 1) for pr in range(NPAIR)])
            run(weave(attn_wave(units, post_round), filler, 6, 7))
            run(proj0[0])
            run(proj_units(1))

    nc.compile()
    return nc


def _get_nc():
    global _BUILT
    if _BUILT is None:
        _BUILT = _build()
    return _BUILT


# ------------------------------------------------- tracing support (axon)

def _ensure_trace_hooks():
    """Register the NTFF profile hook that the bare agent image's antenv
    stub lacks, and neuter the artifact upload (no bucket in-container)."""
    import types
    import concourse.bass_utils as bu

    bu.upload_artifacts = lambda tmpdir: f"local:{tmpdir}"
    try:
        from antenv.axon_hooks import get_axon_ntff_profile_hook  # noqa: F401
        return
    except ImportError:
        pass
    mod = types.ModuleType("antenv.axon_hooks")
    _state = {"hook": None}
    mod.set_axon_ntff_profile_hook = lambda h: _state.__setitem__("hook", h)
    mod.get_axon_ntff_profile_hook = lambda: _state["hook"]
    import antenv
    sys.modules["antenv.axon_hooks"] = mod
    antenv.axon_hooks = mod
    try:
        from trn_agent_boot.trn_boot import _ntff_profile_via_ctypes
        hook = _ntff_profile_via_ctypes("/opt/axon/libaxon_pjrt.so")
        if hook is not None:
            mod.set_axon_ntff_profile_hook(hook)
    except Exception as e:  # pragma: no cover
        print(f"NTFF hook install failed: {e!r}")


# ----------------------------------------------------------------- kernel()

def kernel(x, Wqkv, Wproj, bproj):
    global LAST_RESULT
    x = np.asarray(x, np.float32)
    Wqkv = np.asarray(Wqkv, np.float32)
    Wproj = np.asarray(Wproj, np.float32)
    bproj = np.asarray(bproj, np.float32)
    B = x.shape[0]

    base = _prep_weights(Wqkv, Wproj, bproj)
    bf = ml_dtypes.bfloat16
    in_maps = [
        dict(base, xT=np.ascontiguousarray(x[b].T).astype(bf)) for b in range(B)
    ]
    nc = _get_nc()
    trace = bool(os.environ.get("KBENCH_TRACE"))
    if trace:
        _ensure_trace_hooks()
    res = run_bass_kernel_spmd(
        nc, in_maps, core_ids=list(range(B)), trace=trace)
    LAST_RESULT = res
    out = np.stack([np.asarray(res.results[b]["out"], np.float32).T
                    for b in range(B)])
    return np.ascontiguousarray(out.astype(np.float32))

